# revision 87
# baseline (speedup 1.0000x reference)
"""Trainium2 Bass kernel for BinaryMemoryRNN (scatter_memory).

Math (per batch row b):
    logits = h_prev @ M_w.T + M_b                 [B, 10]
    bits   = (sigmoid(logits) > 0.5) = (logits > -M_b)
    index  = sum(bits * 2^(9-i))                  [B] in [0, 1023]
    h_mem  = memory[index]
    pre    = x @ W_w.T + W_b + h_prev @ U_w.T + U_b + h_mem @ Q_w.T + Q_b
    out    = sigmoid(LayerNorm(pre) * ln_g + ln_b)

Key transforms:
  - h_mem @ Q_w.T == (memory @ Q_w.T)[index]: precompute R = memory @ Q_w.T
    + combined bias ([1024, 1024] bf16) and replace gather+matmul with a row
    gather of R feeding an add.
  - All three big GEMMs (x@W, h@U, memory@Q) run in fp8 (e4m3) with
    DoubleRow perf mode (2 fp8 weights per PE cell, K=256 per matmul,
    measured 2x over bf16). Weights are pre-scaled by SW=128 on the host
    (values ~0.02 would hit e4m3 subnormals). The scale is never undone:
    R is stored at SW-times scale, so PSUM + R-gather stays a plain
    tensor_add, and LayerNorm's scale invariance absorbs SW (only eps is
    scaled by SW^2).
  - The address logits need ~fp32 accuracy (a flipped bit selects a
    completely different memory row): computed from an fp32 copy of h
    read as float32r (fp22, 1-pass PE) in a transposed group scheme
    [10, 512] so the PE cost is moving-dominated, not LDWEIGHTS-dominated.
  - Logits/index run one tile-group ahead of the matmul pipeline; the
    output is written as bf16 and upcast on the host.

Sharding: data-parallel over batch across 8 cores (2048 rows each);
weights + memory table replicated. All operands pre-transposed/tiled on
host so the device does zero transposes:
  - activations as [kp, bt, kc, bp] tiles (K on partitions)
  - weights as [kp, kc, n] (K on partitions, contiguous rhs slices)
"""

import numpy as np
import ml_dtypes
from contextlib import ExitStack

import concourse.bass as bass
import concourse.mybir as mybir
import concourse.tile as tile
from concourse import bacc
from concourse import bass_utils

P = 128            # partitions
NCORES = 8
B = 16384          # full batch
BC = B // NCORES   # batch rows per core (2048)
BT = BC // P       # b-tiles per core (16)
KC = 8             # contraction chunks (1024 / 128)
H = 1024
NB = 10            # address bits
MEM = 1024         # memory rows
LN_EPS = 1e-5
SW = 128.0         # fp8 weight prescale

F32 = mybir.dt.float32
F32R = mybir.dt.float32r
BF16 = mybir.dt.bfloat16
F8 = mybir.dt.float8e4
I32 = mybir.dt.int32
BF16_NP = ml_dtypes.bfloat16
F8_NP = getattr(ml_dtypes, "float8_e4m3", ml_dtypes.float8_e4m3fn)

_CACHE = {}


def _bcast_ap(handle, n):
    """[n] DRAM tensor -> [P, n] AP broadcast across partitions (step 0)."""
    h = handle.ap()
    return bass.AP(tensor=h.tensor, offset=h.offset, ap=[[0, P], *list(h.ap)])


def build_nc(zero_affine=True, warmup=False, r_interleaved=False,
             use_fp8=True):
    nc = bacc.Bacc("TRN2", debug=False, enable_asserts=False,
                   num_devices=NCORES)

    act_dt = F8 if use_fp8 else BF16
    w_dt = F8 if use_fp8 else BF16
    xT = nc.dram_tensor("xT", [P, BT, KC, P], act_dt, kind="ExternalInput")
    hT8 = nc.dram_tensor("hT8", [P, BT, KC, P], act_dt, kind="ExternalInput")
    hTf = nc.dram_tensor("hTf", [P, BT, KC, P], F32R, kind="ExternalInput")
    Wt = nc.dram_tensor("Wt", [P, KC, H], w_dt, kind="ExternalInput")
    Ut = nc.dram_tensor("Ut", [P, KC, H], w_dt, kind="ExternalInput")
    rq_dt = F8 if use_fp8 else BF16
    Qt = nc.dram_tensor("Qt", [P, KC, H], rq_dt, kind="ExternalInput")
    Mem = nc.dram_tensor("Mem", [P, KC, MEM], rq_dt, kind="ExternalInput")
    Mt = nc.dram_tensor("Mt", [P, KC, NB], F32R, kind="ExternalInput")
    cb = nc.dram_tensor("cb", [H], F32, kind="ExternalInput")
    lng = nc.dram_tensor("lng", [H], F32, kind="ExternalInput")
    lnb = nc.dram_tensor("lnb", [H], F32, kind="ExternalInput")
    negmb = nc.dram_tensor("negmb", [NB], F32, kind="ExternalInput")
    powers = nc.dram_tensor("powers", [NB], F32, kind="ExternalInput")
    y = nc.dram_tensor("y", [BC, H], BF16, kind="ExternalOutput")
    R = nc.dram_tensor("Rtab", [MEM, H], BF16, kind="Internal")
    idxd = nc.dram_tensor("idxd", [BT, P], I32, kind="Internal")
    wsink_d = nc.dram_tensor("wsink", [P, 1], F32, kind="Internal")
    y_ap = y.ap()
    R_ap = R.ap()

    GROUP = 4
    DR = mybir.MatmulPerfMode.DoubleRow

    with tile.TileContext(nc) as tc, ExitStack() as ctx:
        wpool = ctx.enter_context(tc.tile_pool(name="weights", bufs=1))
        work = ctx.enter_context(tc.tile_pool(name="work", bufs=6))
        hpool = ctx.enter_context(tc.tile_pool(name="hpool", bufs=2))
        ipool = ctx.enter_context(tc.tile_pool(name="ipool", bufs=2))
        epil = ctx.enter_context(tc.tile_pool(name="epil", bufs=6))
        small = ctx.enter_context(tc.tile_pool(name="small", bufs=2 * GROUP + 2))
        psum = ctx.enter_context(tc.tile_pool(name="psum", bufs=1, space="PSUM"))

        if warmup:
            # ~5us of dummy matmuls on memset data: trips the HAM clock gate
            # to K=8/8 before real matmuls start.
            wu_l = wpool.tile([P, P], BF16)
            wu_r = wpool.tile([P, 512], BF16)
            nc.vector.memset(wu_l[:], 0)
            nc.vector.memset(wu_r[:], 0)
            ps_w = psum.tile([P, 512], F32, tag="psL", space="PSUM", bufs=2)
            for _ in range(12):
                nc.tensor.matmul(out=ps_w[:], lhsT=wu_l[:], rhs=wu_r[:],
                                 start=True, stop=True)
            wsink = wpool.tile([P, 1], F32)
            nc.vector.tensor_copy(out=wsink[:], in_=ps_w[:, 0:1])
            nc.sync.dma_start(out=wsink_d.ap()[:, :], in_=wsink[:])

        # ---- resident constants; mem/q chunked halves so R-phase matmuls
        # start early; w/u/mw on the (idle) vector queue ----
        w_sb = wpool.tile([P, KC, H], w_dt)
        u_sb = wpool.tile([P, KC, H], w_dt)
        q_sb = wpool.tile([P, KC, H], rq_dt)
        mem_sb = wpool.tile([P, KC, MEM], rq_dt)
        mw_sb = wpool.tile([P, KC, NB], F32R)
        # kc-pair quarters; mem on the scalar queue, q on sync, so both
        # streams issue in parallel and the R build starts sooner
        for kcp in range(0, KC, 2):
            nc.scalar.dma_start(out=mem_sb[:, kcp:kcp + 2],
                                in_=Mem.ap()[:, kcp:kcp + 2, :])
            nc.sync.dma_start(out=q_sb[:, kcp:kcp + 2],
                              in_=Qt.ap()[:, kcp:kcp + 2, :])
        nc.sync.dma_start(out=mw_sb[:], in_=Mt.ap()[:, :, :])
        nc.scalar.dma_start(out=w_sb[:], in_=Wt.ap()[:, :, :])
        nc.scalar.dma_start(out=u_sb[:], in_=Ut.ap()[:, :, :])

        cbb = wpool.tile([P, H], F32)
        nc.gpsimd.dma_start(out=cbb[:], in_=_bcast_ap(cb, H))
        if not zero_affine:
            gb = wpool.tile([P, H], F32)
            bb = wpool.tile([P, H], F32)
            nc.gpsimd.dma_start(out=gb[:], in_=_bcast_ap(lng, H))
            nc.gpsimd.dma_start(out=bb[:], in_=_bcast_ap(lnb, H))
        # eps scaled by SW^2: pre lives at SW-times scale in fp8 mode
        eps_val = LN_EPS * (SW * SW if use_fp8 else 1.0)
        eps = wpool.tile([P, 1], F32)
        nc.vector.memset(eps[:], eps_val)
        # transposed-group logits constants
        nmb_c = wpool.tile([NB, 1], F32)
        pw_c = wpool.tile([NB, 1], F32)
        nc.sync.dma_start(out=nmb_c[:], in_=negmb.ap()[:, None])
        nc.sync.dma_start(out=pw_c[:], in_=powers.ap()[:, None])
        ones10 = wpool.tile([NB, 1], BF16)
        nc.vector.memset(ones10[:], 1.0)

        # ---- phase 1: R = memory @ Q_w.T + combined_bias -> DRAM (bf16) ----
        # R table kept at SW-times scale (Q was pre-scaled by SW and the
        # host ships cb*SW): LayerNorm is scale-invariant, so the tile
        # epilogue never needs to unscale -- only eps is scaled by SW^2.
        for mt in range(KC):
            psA = psum.tile([P, 512], F32, tag="psA", space="PSUM", bufs=3)
            psB = psum.tile([P, 512], F32, tag="psB", space="PSUM", bufs=3)
            if use_fp8:
                for kc in range(0, KC, 2):
                    lhs = mem_sb[:, kc:kc + 2, mt * P:(mt + 1) * P]
                    nc.tensor.matmul(out=psA[:], lhsT=lhs,
                                     rhs=q_sb[:, kc:kc + 2, 0:512],
                                     start=(kc == 0), stop=(kc == KC - 2),
                                     perf_mode=DR)
                    nc.tensor.matmul(out=psB[:], lhsT=lhs,
                                     rhs=q_sb[:, kc:kc + 2, 512:1024],
                                     start=(kc == 0), stop=(kc == KC - 2),
                                     perf_mode=DR)
            else:
                for kc in range(KC):
                    lhs = mem_sb[:, kc, mt * P:(mt + 1) * P]
                    nc.tensor.matmul(out=psA[:], lhsT=lhs,
                                     rhs=q_sb[:, kc, 0:512],
                                     start=(kc == 0), stop=(kc == KC - 1))
                    nc.tensor.matmul(out=psB[:], lhsT=lhs,
                                     rhs=q_sb[:, kc, 512:1024],
                                     start=(kc == 0), stop=(kc == KC - 1))
            r_sb = work.tile([P, H], BF16, tag="rtile")
            nc.vector.tensor_add(out=r_sb[:, 0:512], in0=psA[:],
                                 in1=cbb[:, 0:512])
            nc.vector.tensor_add(out=r_sb[:, 512:1024], in0=psB[:],
                                 in1=cbb[:, 512:1024])
            nc.gpsimd.dma_start(out=R_ap[mt * P:(mt + 1) * P, :], in_=r_sb[:])

        # ---- phase 2 ----
        def logits_group(g0, gsz):
            """Transposed fp32r logits for gsz b-tiles -> per-tile idx."""
            n = gsz * P
            hfg = hpool.tile([P, gsz, KC, P], F32R, tag="hfg")
            if gsz > 1:
                h2 = gsz // 2
                nc.sync.dma_start(out=hfg[:, 0:h2],
                                  in_=hTf.ap()[:, g0:g0 + h2, :, :])
                nc.sync.dma_start(out=hfg[:, h2:gsz],
                                  in_=hTf.ap()[:, g0 + h2:g0 + gsz, :, :])
            else:
                nc.sync.dma_start(out=hfg[:],
                                  in_=hTf.ap()[:, g0:g0 + gsz, :, :])
            psLT = psum.tile([NB, 512], F32, tag="psL", space="PSUM",
                             bufs=2)
            for kc in range(KC):
                nc.tensor.matmul(out=psLT[:, 0:n],
                                 lhsT=mw_sb[:, kc, :],
                                 rhs=hfg[:, :, kc, :],
                                 start=(kc == 0), stop=(kc == KC - 1))
            bitsT = ipool.tile([NB, 512], BF16, tag="bitsT")
            nc.vector.tensor_scalar(out=bitsT[:, 0:n], in0=psLT[:, 0:n],
                                    scalar1=nmb_c[:], scalar2=pw_c[:],
                                    op0=mybir.AluOpType.is_gt,
                                    op1=mybir.AluOpType.mult)
            psI = psum.tile([1, 512], F32, tag="psL", space="PSUM", bufs=2)
            nc.tensor.matmul(out=psI[0:1, 0:n], lhsT=ones10[:],
                             rhs=bitsT[:, 0:n],
                             start=True, stop=True)
            # transpose [1, n] -> [P, gsz] via a DRAM round trip with a
            # rearranged read AP (replaces gsz PE transposes + copies)
            idxT = ipool.tile([1, 512], I32, tag="idxT")
            nc.vector.tensor_copy(out=idxT[0:1, 0:n], in_=psI[0:1, 0:n])
            # on the gpsimd queue (with the gathers they feed), keeping the
            # sync queue free for hfg/x/h streaming
            nc.gpsimd.dma_start(out=idxd.ap()[g0:g0 + gsz, :],
                                in_=idxT[0:1, 0:n])
            idxP = small.tile([P, gsz], I32, tag="idxP")
            h_ap = idxd.ap()
            nc.gpsimd.dma_start(
                out=idxP[:],
                in_=bass.AP(tensor=h_ap.tensor, offset=g0 * P,
                            ap=[[1, P], [P, gsz]]))
            return [idxP[:, tb:tb + 1] for tb in range(gsz)]

        def stage_a(bt, idx_ap):
            xb = work.tile([P, KC, P], act_dt, tag="xb")
            hb = work.tile([P, KC, P], act_dt, tag="hb")
            nc.sync.dma_start(out=xb[:], in_=xT.ap()[:, bt, :, :])
            nc.sync.dma_start(out=hb[:], in_=hT8.ap()[:, bt, :, :])

            ps0 = psum.tile([P, 512], F32, tag="psA", space="PSUM", bufs=3)
            ps1 = psum.tile([P, 512], F32, tag="psB", space="PSUM", bufs=3)

            rg = work.tile([P, H], BF16, tag="rg")
            nc.gpsimd.indirect_dma_start(
                out=rg[:], out_offset=None, in_=R_ap[:, :],
                in_offset=bass.IndirectOffsetOnAxis(ap=idx_ap, axis=0))

            if use_fp8:
                for kc in range(0, KC, 2):
                    nc.tensor.matmul(out=ps0[:], lhsT=xb[:, kc:kc + 2, :],
                                     rhs=w_sb[:, kc:kc + 2, 0:512],
                                     start=(kc == 0), stop=False, perf_mode=DR)
                    nc.tensor.matmul(out=ps1[:], lhsT=xb[:, kc:kc + 2, :],
                                     rhs=w_sb[:, kc:kc + 2, 512:1024],
                                     start=(kc == 0), stop=False, perf_mode=DR)
                for kc in range(0, KC, 2):
                    nc.tensor.matmul(out=ps0[:], lhsT=hb[:, kc:kc + 2, :],
                                     rhs=u_sb[:, kc:kc + 2, 0:512],
                                     start=False, stop=(kc == KC - 2),
                                     perf_mode=DR)
                    nc.tensor.matmul(out=ps1[:], lhsT=hb[:, kc:kc + 2, :],
                                     rhs=u_sb[:, kc:kc + 2, 512:1024],
                                     start=False, stop=(kc == KC - 2),
                                     perf_mode=DR)
            else:
                for kc in range(KC):
                    nc.tensor.matmul(out=ps0[:], lhsT=xb[:, kc, :],
                                     rhs=w_sb[:, kc, 0:512],
                                     start=(kc == 0), stop=False)
                    nc.tensor.matmul(out=ps1[:], lhsT=xb[:, kc, :],
                                     rhs=w_sb[:, kc, 512:1024],
                                     start=(kc == 0), stop=False)
                for kc in range(KC):
                    nc.tensor.matmul(out=ps0[:], lhsT=hb[:, kc, :],
                                     rhs=u_sb[:, kc, 0:512],
                                     start=False, stop=(kc == KC - 1))
                    nc.tensor.matmul(out=ps1[:], lhsT=hb[:, kc, :],
                                     rhs=u_sb[:, kc, 512:1024],
                                     start=False, stop=(kc == KC - 1))

            pre = epil.tile([P, 2, 512], F32, tag="pre")
            nc.vector.tensor_add(out=pre[:, 0], in0=ps0[:],
                                 in1=rg[:, 0:512])
            nc.vector.tensor_add(out=pre[:, 1], in0=ps1[:],
                                 in1=rg[:, 512:1024])

            stats = small.tile([P, 2, 6], F32, tag="stats")
            mv = small.tile([P, 2], F32, tag="mv")
            nc.vector.bn_stats(out=stats[:, 0, :], in_=pre[:, 0])
            nc.vector.bn_stats(out=stats[:, 1, :], in_=pre[:, 1])
            nc.vector.bn_aggr(out=mv[:], in_=stats[:])

            if zero_affine:
                v = small.tile([P, 1], F32, tag="v")
                ri = small.tile([P, 1], I32, tag="ri")
                t = small.tile([P, 1], F32, tag="t")
                nmr = small.tile([P, 1], F32, tag="nmr")
                ry = ri[:].bitcast(F32)
                nc.vector.tensor_scalar_add(out=v[:], in0=mv[:, 1:2],
                                            scalar1=eps_val)
                nc.vector.tensor_scalar(out=ri[:], in0=v[:].bitcast(I32),
                                        scalar1=1, scalar2=None,
                                        op0=mybir.AluOpType.arith_shift_right)
                nc.vector.tensor_scalar(out=ri[:], in0=ri[:], scalar1=0,
                                        scalar2=None,
                                        op0=mybir.AluOpType.bitwise_not)
                nc.vector.tensor_scalar(out=ri[:], in0=ri[:],
                                        scalar1=0x5F3759E0, scalar2=None,
                                        op0=mybir.AluOpType.add)
                # one Newton step on the magic-constant estimate gives rstd
                # to ~0.2% -- far below the fp8 matmul noise
                for _ in range(1):
                    nc.vector.tensor_tensor(out=t[:], in0=ry, in1=ry,
                                            op=mybir.AluOpType.mult)
                    nc.vector.tensor_tensor(out=t[:], in0=t[:], in1=v[:],
                                            op=mybir.AluOpType.mult)
                    nc.vector.tensor_scalar(out=t[:], in0=t[:], scalar1=-0.5,
                                            scalar2=1.5,
                                            op0=mybir.AluOpType.mult,
                                            op1=mybir.AluOpType.add)
                    nc.vector.tensor_tensor(out=ry, in0=ry, in1=t[:],
                                            op=mybir.AluOpType.mult)
                nc.vector.scalar_tensor_tensor(out=nmr[:], in0=mv[:, 0:1],
                                               scalar=-1.0, in1=ry,
                                               op0=mybir.AluOpType.mult,
                                               op1=mybir.AluOpType.mult)
                ob = work.tile([P, H], BF16, tag="ob")
                nc.scalar.activation(out=ob[:], in_=pre[:],
                                     func=mybir.ActivationFunctionType.Sigmoid,
                                     bias=nmr[:], scale=ri[:].bitcast(F32))
                nc.scalar.dma_start(out=y_ap[bt * P:(bt + 1) * P, :], in_=ob[:])
                return None

            sd = small.tile([P, 1], F32, tag="sd")
            rstd = small.tile([P, 1], F32, tag="rstd")
            nc.scalar.activation(out=sd[:], in_=mv[:, 1:2],
                                 func=mybir.ActivationFunctionType.Sqrt,
                                 bias=eps[:], scale=1.0)
            nc.vector.reciprocal(out=rstd[:], in_=sd[:])
            return pre, mv, rstd

        def stage_b(bt, pre, mv, rstd):
            nc.vector.scalar_tensor_tensor(out=pre[:], in0=pre[:],
                                           scalar=mv[:, 0:1], in1=gb[:],
                                           op0=mybir.AluOpType.subtract,
                                           op1=mybir.AluOpType.mult)
            nc.vector.scalar_tensor_tensor(out=pre[:], in0=pre[:],
                                           scalar=rstd[:], in1=bb[:],
                                           op0=mybir.AluOpType.mult,
                                           op1=mybir.AluOpType.add)
            ob = work.tile([P, H], BF16, tag="ob")
            nc.scalar.activation(out=ob[:], in_=pre[:],
                                 func=mybir.ActivationFunctionType.Sigmoid)
            nc.scalar.dma_start(out=y_ap[bt * P:(bt + 1) * P, :], in_=ob[:])

        # logits run one group ahead of the matmul tiles so the final
        # group's drain is just tile epilogues, not the full index chain.
        # the last two groups are small so the pipeline drain is short.
        groups = [(0, 4), (4, 4), (8, 4), (12, 2), (14, 2)]
        pend = logits_group(*groups[0])
        for gi, (g0, gsz) in enumerate(groups):
            idxs = pend
            if gi + 1 < len(groups):
                pend = logits_group(*groups[gi + 1])
            if zero_affine:
                for tb in range(gsz):
                    stage_a(g0 + tb, idxs[tb])
            else:
                staged = [(g0 + tb, *stage_a(g0 + tb, idxs[tb]))
                          for tb in range(gsz)]
                for bt, pre, mv, rstd in staged:
                    stage_b(bt, pre, mv, rstd)

    nc.compile()
    return nc


import os as _os

FLAGS = {
    "warmup": bool(int(_os.environ.get("K_WARMUP", "1"))),
    "r_interleaved": bool(int(_os.environ.get("K_RINT", "0"))),
    "use_fp8": bool(int(_os.environ.get("K_FP8", "1"))),
}


def _get_nc(zero_affine=True):
    key = ("nc", zero_affine, tuple(sorted(FLAGS.items())))
    if key not in _CACHE:
        _CACHE[key] = build_nc(zero_affine, **FLAGS)
    return _CACHE[key]


def _tile_act(a):
    """[BC, 1024] -> [kp, bt, kc, bp] = a[bt*128+bp, kc*128+kp]."""
    return np.ascontiguousarray(
        a.reshape(BT, P, KC, P).transpose(3, 0, 2, 1))


def _tile_w(w):
    """[n, 1024] (n = out dim, contraction on axis 1) -> [kp, kc, n]."""
    return np.ascontiguousarray(w.T.reshape(KC, P, -1).transpose(1, 0, 2))


def _f8(a):
    return np.clip(a, -240.0, 240.0).astype(F8_NP)


def prepare_in_maps(inputs, use_fp8=True):
    x = np.asarray(inputs["x"], np.float32)
    h = np.asarray(inputs["h_prev"], np.float32)
    memory = np.asarray(inputs["memory"], np.float32)
    W_w = np.asarray(inputs["W_w"], np.float32)
    U_w = np.asarray(inputs["U_w"], np.float32)
    Q_w = np.asarray(inputs["Q_w"], np.float32)
    M_w = np.asarray(inputs["M_w"], np.float32)
    W_b = np.asarray(inputs["W_b"], np.float32)
    U_b = np.asarray(inputs["U_b"], np.float32)
    Q_b = np.asarray(inputs["Q_b"], np.float32)
    M_b = np.asarray(inputs["M_b"], np.float32)
    ln_g = np.asarray(inputs["ln_g"], np.float32)
    ln_b = np.asarray(inputs["ln_b"], np.float32)

    if use_fp8:
        Wt_h = _f8(_tile_w(W_w) * SW)
        Ut_h = _f8(_tile_w(U_w) * SW)
    else:
        Wt_h = _tile_w(W_w).astype(BF16_NP)
        Ut_h = _tile_w(U_w).astype(BF16_NP)

    # memory contraction for R = memory @ Q_w.T is over memory's axis 1
    # (HIDDEN); rows (axis 0) are the "out" dim -> same transform as W.
    if use_fp8:
        Qt_h = _f8(_tile_w(Q_w) * SW)
        mem_t = _f8(_tile_w(memory))
    else:
        Qt_h = _tile_w(Q_w).astype(BF16_NP)
        mem_t = _tile_w(memory).astype(BF16_NP)
    shared = {
        "Wt": Wt_h,
        "Ut": Ut_h,
        "Qt": Qt_h,
        "Mem": mem_t,
        "Mt": _tile_w(M_w).astype(np.float32),
        "cb": np.ascontiguousarray((W_b + U_b + Q_b)
                                   * (SW if use_fp8 else 1.0)),
        "lng": np.ascontiguousarray(ln_g),
        "lnb": np.ascontiguousarray(ln_b),
        "negmb": np.ascontiguousarray(-M_b),
        "powers": (2.0 ** np.arange(NB - 1, -1, -1)).astype(np.float32),
    }
    in_maps = []
    for i in range(NCORES):
        sl = slice(i * BC, (i + 1) * BC)
        xt = _tile_act(x[sl])
        ht = _tile_act(h[sl])
        m = dict(shared)
        m["xT"] = _f8(xt) if use_fp8 else xt.astype(BF16_NP)
        m["hT8"] = _f8(ht) if use_fp8 else ht.astype(BF16_NP)
        m["hTf"] = ht  # fp32 copy for the address logits
        in_maps.append(m)
    return in_maps


def run(inputs, trace=False, trace_cores=None):
    zero_affine = bool(
        np.all(np.asarray(inputs["ln_g"], np.float32) == 1.0)
        and np.all(np.asarray(inputs["ln_b"], np.float32) == 0.0))
    nc = _get_nc(zero_affine)
    in_maps = prepare_in_maps(inputs, use_fp8=FLAGS["use_fp8"])
    res = bass_utils.run_bass_kernel_spmd(
        nc, in_maps, core_ids=list(range(NCORES)), trace=trace,
        trace_cores=trace_cores)
    out = np.concatenate([np.asarray(r["y"], np.float32)
                          for r in res.results], axis=0)
    return out, res


def kernel(**inputs):
    out, _ = run(inputs)
    return out.astype(np.float32)


def enable_profiling():
    """Inject the missing antenv.axon_hooks shim so trace=True works, and
    neutralize the S3 artifact upload (zero-egress container)."""
    import sys
    import types
    try:
        import antenv.axon_hooks  # noqa: F401
    except ImportError:
        mod = types.ModuleType("antenv.axon_hooks")
        _hook = [None]
        mod.set_axon_ntff_profile_hook = lambda h: _hook.__setitem__(0, h)
        mod.get_axon_ntff_profile_hook = lambda: _hook[0]
        sys.modules["antenv.axon_hooks"] = mod
        from trn_agent_boot.trn_boot import _ntff_profile_via_ctypes
        mod.set_axon_ntff_profile_hook(
            _ntff_profile_via_ctypes("/opt/axon/libaxon_pjrt.so"))
    bass_utils.upload_artifacts = lambda d: "local://" + str(d)


# revision 88
# speedup vs baseline: 1.0080x; 1.0080x over previous
"""Trainium2 Bass kernel for BinaryMemoryRNN (scatter_memory).

Math (per batch row b):
    logits = h_prev @ M_w.T + M_b                 [B, 10]
    bits   = (sigmoid(logits) > 0.5) = (logits > -M_b)
    index  = sum(bits * 2^(9-i))                  [B] in [0, 1023]
    h_mem  = memory[index]
    pre    = x @ W_w.T + W_b + h_prev @ U_w.T + U_b + h_mem @ Q_w.T + Q_b
    out    = sigmoid(LayerNorm(pre) * ln_g + ln_b)

Key transforms:
  - h_mem @ Q_w.T == (memory @ Q_w.T)[index]: precompute R = memory @ Q_w.T
    + combined bias ([1024, 1024] bf16) and replace gather+matmul with a row
    gather of R feeding an add.
  - All three big GEMMs (x@W, h@U, memory@Q) run in fp8 (e4m3) with
    DoubleRow perf mode (2 fp8 weights per PE cell, K=256 per matmul,
    measured 2x over bf16). Weights are pre-scaled by SW=128 on the host
    (values ~0.02 would hit e4m3 subnormals). The scale is never undone:
    R is stored at SW-times scale, so PSUM + R-gather stays a plain
    tensor_add, and LayerNorm's scale invariance absorbs SW (only eps is
    scaled by SW^2).
  - The address logits need ~fp32 accuracy (a flipped bit selects a
    completely different memory row): computed from an fp32 copy of h
    read as float32r (fp22, 1-pass PE) in a transposed group scheme
    [10, 512] so the PE cost is moving-dominated, not LDWEIGHTS-dominated.
  - Logits/index run one tile-group ahead of the matmul pipeline; the
    output is written as bf16 and upcast on the host.

Sharding: data-parallel over batch across 8 cores (2048 rows each);
weights + memory table replicated. All operands pre-transposed/tiled on
host so the device does zero transposes:
  - activations as [kp, bt, kc, bp] tiles (K on partitions)
  - weights as [kp, kc, n] (K on partitions, contiguous rhs slices)
"""

import numpy as np
import ml_dtypes
from contextlib import ExitStack

import concourse.bass as bass
import concourse.mybir as mybir
import concourse.tile as tile
from concourse import bacc
from concourse import bass_utils

P = 128            # partitions
NCORES = 8
B = 16384          # full batch
BC = B // NCORES   # batch rows per core (2048)
BT = BC // P       # b-tiles per core (16)
KC = 8             # contraction chunks (1024 / 128)
H = 1024
NB = 10            # address bits
MEM = 1024         # memory rows
LN_EPS = 1e-5
SW = 128.0         # fp8 weight prescale

F32 = mybir.dt.float32
F32R = mybir.dt.float32r
BF16 = mybir.dt.bfloat16
F8 = mybir.dt.float8e4
I32 = mybir.dt.int32
BF16_NP = ml_dtypes.bfloat16
F8_NP = getattr(ml_dtypes, "float8_e4m3", ml_dtypes.float8_e4m3fn)

_CACHE = {}


def _bcast_ap(handle, n):
    """[n] DRAM tensor -> [P, n] AP broadcast across partitions (step 0)."""
    h = handle.ap()
    return bass.AP(tensor=h.tensor, offset=h.offset, ap=[[0, P], *list(h.ap)])


def build_nc(zero_affine=True, warmup=False, r_interleaved=False,
             use_fp8=True):
    nc = bacc.Bacc("TRN2", debug=False, enable_asserts=False,
                   num_devices=NCORES)

    act_dt = F8 if use_fp8 else BF16
    w_dt = F8 if use_fp8 else BF16
    xT = nc.dram_tensor("xT", [P, BT, KC, P], act_dt, kind="ExternalInput")
    hT8 = nc.dram_tensor("hT8", [P, BT, KC, P], act_dt, kind="ExternalInput")
    hTf = nc.dram_tensor("hTf", [P, BT, KC, P], F32R, kind="ExternalInput")
    Wt = nc.dram_tensor("Wt", [P, KC, H], w_dt, kind="ExternalInput")
    Ut = nc.dram_tensor("Ut", [P, KC, H], w_dt, kind="ExternalInput")
    rq_dt = F8 if use_fp8 else BF16
    Qt = nc.dram_tensor("Qt", [P, KC, H], rq_dt, kind="ExternalInput")
    Mem = nc.dram_tensor("Mem", [P, KC, MEM], rq_dt, kind="ExternalInput")
    Mt = nc.dram_tensor("Mt", [P, KC, NB], F32R, kind="ExternalInput")
    cb = nc.dram_tensor("cb", [H], F32, kind="ExternalInput")
    lng = nc.dram_tensor("lng", [H], F32, kind="ExternalInput")
    lnb = nc.dram_tensor("lnb", [H], F32, kind="ExternalInput")
    negmb = nc.dram_tensor("negmb", [NB], F32, kind="ExternalInput")
    powers = nc.dram_tensor("powers", [NB], F32, kind="ExternalInput")
    y = nc.dram_tensor("y", [BC, H], BF16, kind="ExternalOutput")
    R = nc.dram_tensor("Rtab", [MEM, H], BF16, kind="Internal")
    idxd = nc.dram_tensor("idxd", [BT, P], I32, kind="Internal")
    wsink_d = nc.dram_tensor("wsink", [P, 1], F32, kind="Internal")
    y_ap = y.ap()
    R_ap = R.ap()

    GROUP = 4
    DR = mybir.MatmulPerfMode.DoubleRow

    with tile.TileContext(nc) as tc, ExitStack() as ctx:
        wpool = ctx.enter_context(tc.tile_pool(name="weights", bufs=1))
        work = ctx.enter_context(tc.tile_pool(name="work", bufs=6))
        hpool = ctx.enter_context(tc.tile_pool(name="hpool", bufs=2))
        ipool = ctx.enter_context(tc.tile_pool(name="ipool", bufs=2))
        epil = ctx.enter_context(tc.tile_pool(name="epil", bufs=6))
        small = ctx.enter_context(tc.tile_pool(name="small", bufs=2 * GROUP + 2))
        psum = ctx.enter_context(tc.tile_pool(name="psum", bufs=1, space="PSUM"))

        if warmup:
            # ~5us of dummy matmuls on memset data: trips the HAM clock gate
            # to K=8/8 before real matmuls start.
            wu_l = wpool.tile([P, P], BF16)
            wu_r = wpool.tile([P, 512], BF16)
            nc.vector.memset(wu_l[:], 0)
            nc.vector.memset(wu_r[:], 0)
            ps_w = psum.tile([P, 512], F32, tag="psL", space="PSUM", bufs=2)
            for _ in range(12):
                nc.tensor.matmul(out=ps_w[:], lhsT=wu_l[:], rhs=wu_r[:],
                                 start=True, stop=True)
            wsink = wpool.tile([P, 1], F32)
            nc.vector.tensor_copy(out=wsink[:], in_=ps_w[:, 0:1])
            nc.sync.dma_start(out=wsink_d.ap()[:, :], in_=wsink[:])

        # ---- resident constants; mem/q chunked halves so R-phase matmuls
        # start early; w/u/mw on the (idle) vector queue ----
        w_sb = wpool.tile([P, KC, H], w_dt)
        u_sb = wpool.tile([P, KC, H], w_dt)
        q_sb = wpool.tile([P, KC, H], rq_dt)
        mem_sb = wpool.tile([P, KC, MEM], rq_dt)
        mw_sb = wpool.tile([P, KC, NB], F32R)
        # kc-pair quarters; mem on the scalar queue, q on sync, so both
        # streams issue in parallel and the R build starts sooner
        for kcp in range(0, KC, 2):
            nc.scalar.dma_start(out=mem_sb[:, kcp:kcp + 2],
                                in_=Mem.ap()[:, kcp:kcp + 2, :])
            nc.sync.dma_start(out=q_sb[:, kcp:kcp + 2],
                              in_=Qt.ap()[:, kcp:kcp + 2, :])
        nc.sync.dma_start(out=mw_sb[:], in_=Mt.ap()[:, :, :])
        nc.scalar.dma_start(out=w_sb[:], in_=Wt.ap()[:, :, :])
        nc.scalar.dma_start(out=u_sb[:], in_=Ut.ap()[:, :, :])

        cbb = wpool.tile([P, H], F32)
        nc.gpsimd.dma_start(out=cbb[:], in_=_bcast_ap(cb, H))
        if not zero_affine:
            gb = wpool.tile([P, H], F32)
            bb = wpool.tile([P, H], F32)
            nc.gpsimd.dma_start(out=gb[:], in_=_bcast_ap(lng, H))
            nc.gpsimd.dma_start(out=bb[:], in_=_bcast_ap(lnb, H))
        # eps scaled by SW^2: pre lives at SW-times scale in fp8 mode
        eps_val = LN_EPS * (SW * SW if use_fp8 else 1.0)
        eps = wpool.tile([P, 1], F32)
        nc.vector.memset(eps[:], eps_val)
        # transposed-group logits constants
        nmb_c = wpool.tile([NB, 1], F32)
        pw_c = wpool.tile([NB, 1], F32)
        nc.sync.dma_start(out=nmb_c[:], in_=negmb.ap()[:, None])
        nc.sync.dma_start(out=pw_c[:], in_=powers.ap()[:, None])
        ones10 = wpool.tile([NB, 1], BF16)
        nc.vector.memset(ones10[:], 1.0)

        # ---- phase 1: R = memory @ Q_w.T + combined_bias -> DRAM (bf16) ----
        # R table kept at SW-times scale (Q was pre-scaled by SW and the
        # host ships cb*SW): LayerNorm is scale-invariant, so the tile
        # epilogue never needs to unscale -- only eps is scaled by SW^2.
        for mt in range(KC):
            psA = psum.tile([P, 512], F32, tag="psA", space="PSUM", bufs=3)
            psB = psum.tile([P, 512], F32, tag="psB", space="PSUM", bufs=3)
            if use_fp8:
                for kc in range(0, KC, 2):
                    lhs = mem_sb[:, kc:kc + 2, mt * P:(mt + 1) * P]
                    nc.tensor.matmul(out=psA[:], lhsT=lhs,
                                     rhs=q_sb[:, kc:kc + 2, 0:512],
                                     start=(kc == 0), stop=(kc == KC - 2),
                                     perf_mode=DR)
                    nc.tensor.matmul(out=psB[:], lhsT=lhs,
                                     rhs=q_sb[:, kc:kc + 2, 512:1024],
                                     start=(kc == 0), stop=(kc == KC - 2),
                                     perf_mode=DR)
            else:
                for kc in range(KC):
                    lhs = mem_sb[:, kc, mt * P:(mt + 1) * P]
                    nc.tensor.matmul(out=psA[:], lhsT=lhs,
                                     rhs=q_sb[:, kc, 0:512],
                                     start=(kc == 0), stop=(kc == KC - 1))
                    nc.tensor.matmul(out=psB[:], lhsT=lhs,
                                     rhs=q_sb[:, kc, 512:1024],
                                     start=(kc == 0), stop=(kc == KC - 1))
            r_sb = work.tile([P, H], BF16, tag="rtile")
            nc.vector.tensor_add(out=r_sb[:, 0:512], in0=psA[:],
                                 in1=cbb[:, 0:512])
            nc.vector.tensor_add(out=r_sb[:, 512:1024], in0=psB[:],
                                 in1=cbb[:, 512:1024])
            nc.gpsimd.dma_start(out=R_ap[mt * P:(mt + 1) * P, :], in_=r_sb[:])

        # ---- phase 2 ----
        def logits_group(g0, gsz):
            """Transposed fp32r logits for gsz b-tiles -> per-tile idx."""
            n = gsz * P
            hfg = hpool.tile([P, gsz, KC, P], F32R, tag="hfg")
            if gsz > 1:
                h2 = gsz // 2
                nc.sync.dma_start(out=hfg[:, 0:h2],
                                  in_=hTf.ap()[:, g0:g0 + h2, :, :])
                nc.sync.dma_start(out=hfg[:, h2:gsz],
                                  in_=hTf.ap()[:, g0 + h2:g0 + gsz, :, :])
            else:
                nc.sync.dma_start(out=hfg[:],
                                  in_=hTf.ap()[:, g0:g0 + gsz, :, :])
            psLT = psum.tile([NB, 512], F32, tag="psL", space="PSUM",
                             bufs=2)
            for kc in range(KC):
                nc.tensor.matmul(out=psLT[:, 0:n],
                                 lhsT=mw_sb[:, kc, :],
                                 rhs=hfg[:, :, kc, :],
                                 start=(kc == 0), stop=(kc == KC - 1))
            bitsT = ipool.tile([NB, 512], BF16, tag="bitsT")
            nc.vector.tensor_scalar(out=bitsT[:, 0:n], in0=psLT[:, 0:n],
                                    scalar1=nmb_c[:], scalar2=pw_c[:],
                                    op0=mybir.AluOpType.is_gt,
                                    op1=mybir.AluOpType.mult)
            psI = psum.tile([1, 512], F32, tag="psL", space="PSUM", bufs=2)
            nc.tensor.matmul(out=psI[0:1, 0:n], lhsT=ones10[:],
                             rhs=bitsT[:, 0:n],
                             start=True, stop=True)
            # transpose [1, n] -> [P, gsz] via a DRAM round trip with a
            # rearranged read AP (replaces gsz PE transposes + copies)
            idxT = ipool.tile([1, 512], I32, tag="idxT")
            nc.vector.tensor_copy(out=idxT[0:1, 0:n], in_=psI[0:1, 0:n])
            nc.sync.dma_start(out=idxd.ap()[g0:g0 + gsz, :],
                              in_=idxT[0:1, 0:n])
            idxP = small.tile([P, gsz], I32, tag="idxP")
            h_ap = idxd.ap()
            nc.sync.dma_start(
                out=idxP[:],
                in_=bass.AP(tensor=h_ap.tensor, offset=g0 * P,
                            ap=[[1, P], [P, gsz]]))
            return [idxP[:, tb:tb + 1] for tb in range(gsz)]

        def stage_a(bt, idx_ap):
            xb = work.tile([P, KC, P], act_dt, tag="xb")
            hb = work.tile([P, KC, P], act_dt, tag="hb")
            nc.sync.dma_start(out=xb[:], in_=xT.ap()[:, bt, :, :])
            nc.sync.dma_start(out=hb[:], in_=hT8.ap()[:, bt, :, :])

            ps0 = psum.tile([P, 512], F32, tag="psA", space="PSUM", bufs=3)
            ps1 = psum.tile([P, 512], F32, tag="psB", space="PSUM", bufs=3)

            rg = work.tile([P, H], BF16, tag="rg")
            nc.gpsimd.indirect_dma_start(
                out=rg[:], out_offset=None, in_=R_ap[:, :],
                in_offset=bass.IndirectOffsetOnAxis(ap=idx_ap, axis=0))

            if use_fp8:
                for kc in range(0, KC, 2):
                    nc.tensor.matmul(out=ps0[:], lhsT=xb[:, kc:kc + 2, :],
                                     rhs=w_sb[:, kc:kc + 2, 0:512],
                                     start=(kc == 0), stop=False, perf_mode=DR)
                    nc.tensor.matmul(out=ps1[:], lhsT=xb[:, kc:kc + 2, :],
                                     rhs=w_sb[:, kc:kc + 2, 512:1024],
                                     start=(kc == 0), stop=False, perf_mode=DR)
                for kc in range(0, KC, 2):
                    nc.tensor.matmul(out=ps0[:], lhsT=hb[:, kc:kc + 2, :],
                                     rhs=u_sb[:, kc:kc + 2, 0:512],
                                     start=False, stop=(kc == KC - 2),
                                     perf_mode=DR)
                    nc.tensor.matmul(out=ps1[:], lhsT=hb[:, kc:kc + 2, :],
                                     rhs=u_sb[:, kc:kc + 2, 512:1024],
                                     start=False, stop=(kc == KC - 2),
                                     perf_mode=DR)
            else:
                for kc in range(KC):
                    nc.tensor.matmul(out=ps0[:], lhsT=xb[:, kc, :],
                                     rhs=w_sb[:, kc, 0:512],
                                     start=(kc == 0), stop=False)
                    nc.tensor.matmul(out=ps1[:], lhsT=xb[:, kc, :],
                                     rhs=w_sb[:, kc, 512:1024],
                                     start=(kc == 0), stop=False)
                for kc in range(KC):
                    nc.tensor.matmul(out=ps0[:], lhsT=hb[:, kc, :],
                                     rhs=u_sb[:, kc, 0:512],
                                     start=False, stop=(kc == KC - 1))
                    nc.tensor.matmul(out=ps1[:], lhsT=hb[:, kc, :],
                                     rhs=u_sb[:, kc, 512:1024],
                                     start=False, stop=(kc == KC - 1))

            pre = epil.tile([P, 2, 512], F32, tag="pre")
            nc.vector.tensor_add(out=pre[:, 0], in0=ps0[:],
                                 in1=rg[:, 0:512])
            nc.vector.tensor_add(out=pre[:, 1], in0=ps1[:],
                                 in1=rg[:, 512:1024])

            stats = small.tile([P, 2, 6], F32, tag="stats")
            mv = small.tile([P, 2], F32, tag="mv")
            nc.vector.bn_stats(out=stats[:, 0, :], in_=pre[:, 0])
            nc.vector.bn_stats(out=stats[:, 1, :], in_=pre[:, 1])
            nc.vector.bn_aggr(out=mv[:], in_=stats[:])

            if zero_affine:
                v = small.tile([P, 1], F32, tag="v")
                ri = small.tile([P, 1], I32, tag="ri")
                t = small.tile([P, 1], F32, tag="t")
                nmr = small.tile([P, 1], F32, tag="nmr")
                ry = ri[:].bitcast(F32)
                nc.vector.tensor_scalar_add(out=v[:], in0=mv[:, 1:2],
                                            scalar1=eps_val)
                nc.vector.tensor_scalar(out=ri[:], in0=v[:].bitcast(I32),
                                        scalar1=1, scalar2=None,
                                        op0=mybir.AluOpType.arith_shift_right)
                nc.vector.tensor_scalar(out=ri[:], in0=ri[:], scalar1=0,
                                        scalar2=None,
                                        op0=mybir.AluOpType.bitwise_not)
                nc.vector.tensor_scalar(out=ri[:], in0=ri[:],
                                        scalar1=0x5F3759E0, scalar2=None,
                                        op0=mybir.AluOpType.add)
                # one Newton step on the magic-constant estimate gives rstd
                # to ~0.2% -- far below the fp8 matmul noise
                for _ in range(1):
                    nc.vector.tensor_tensor(out=t[:], in0=ry, in1=ry,
                                            op=mybir.AluOpType.mult)
                    nc.vector.tensor_tensor(out=t[:], in0=t[:], in1=v[:],
                                            op=mybir.AluOpType.mult)
                    nc.vector.tensor_scalar(out=t[:], in0=t[:], scalar1=-0.5,
                                            scalar2=1.5,
                                            op0=mybir.AluOpType.mult,
                                            op1=mybir.AluOpType.add)
                    nc.vector.tensor_tensor(out=ry, in0=ry, in1=t[:],
                                            op=mybir.AluOpType.mult)
                nc.vector.scalar_tensor_tensor(out=nmr[:], in0=mv[:, 0:1],
                                               scalar=-1.0, in1=ry,
                                               op0=mybir.AluOpType.mult,
                                               op1=mybir.AluOpType.mult)
                ob = work.tile([P, H], BF16, tag="ob")
                nc.scalar.activation(out=ob[:], in_=pre[:],
                                     func=mybir.ActivationFunctionType.Sigmoid,
                                     bias=nmr[:], scale=ri[:].bitcast(F32))
                nc.scalar.dma_start(out=y_ap[bt * P:(bt + 1) * P, :], in_=ob[:])
                return None

            sd = small.tile([P, 1], F32, tag="sd")
            rstd = small.tile([P, 1], F32, tag="rstd")
            nc.scalar.activation(out=sd[:], in_=mv[:, 1:2],
                                 func=mybir.ActivationFunctionType.Sqrt,
                                 bias=eps[:], scale=1.0)
            nc.vector.reciprocal(out=rstd[:], in_=sd[:])
            return pre, mv, rstd

        def stage_b(bt, pre, mv, rstd):
            nc.vector.scalar_tensor_tensor(out=pre[:], in0=pre[:],
                                           scalar=mv[:, 0:1], in1=gb[:],
                                           op0=mybir.AluOpType.subtract,
                                           op1=mybir.AluOpType.mult)
            nc.vector.scalar_tensor_tensor(out=pre[:], in0=pre[:],
                                           scalar=rstd[:], in1=bb[:],
                                           op0=mybir.AluOpType.mult,
                                           op1=mybir.AluOpType.add)
            ob = work.tile([P, H], BF16, tag="ob")
            nc.scalar.activation(out=ob[:], in_=pre[:],
                                 func=mybir.ActivationFunctionType.Sigmoid)
            nc.scalar.dma_start(out=y_ap[bt * P:(bt + 1) * P, :], in_=ob[:])

        # logits run one group ahead of the matmul tiles so the final
        # group's drain is just tile epilogues, not the full index chain.
        # the last two groups are small so the pipeline drain is short.
        groups = [(0, 4), (4, 4), (8, 4), (12, 2), (14, 2)]
        pend = logits_group(*groups[0])
        for gi, (g0, gsz) in enumerate(groups):
            idxs = pend
            if gi + 1 < len(groups):
                pend = logits_group(*groups[gi + 1])
            if zero_affine:
                for tb in range(gsz):
                    stage_a(g0 + tb, idxs[tb])
            else:
                staged = [(g0 + tb, *stage_a(g0 + tb, idxs[tb]))
                          for tb in range(gsz)]
                for bt, pre, mv, rstd in staged:
                    stage_b(bt, pre, mv, rstd)

    nc.compile()
    return nc


import os as _os

FLAGS = {
    "warmup": bool(int(_os.environ.get("K_WARMUP", "1"))),
    "r_interleaved": bool(int(_os.environ.get("K_RINT", "0"))),
    "use_fp8": bool(int(_os.environ.get("K_FP8", "1"))),
}


def _get_nc(zero_affine=True):
    key = ("nc", zero_affine, tuple(sorted(FLAGS.items())))
    if key not in _CACHE:
        _CACHE[key] = build_nc(zero_affine, **FLAGS)
    return _CACHE[key]


def _tile_act(a):
    """[BC, 1024] -> [kp, bt, kc, bp] = a[bt*128+bp, kc*128+kp]."""
    return np.ascontiguousarray(
        a.reshape(BT, P, KC, P).transpose(3, 0, 2, 1))


def _tile_w(w):
    """[n, 1024] (n = out dim, contraction on axis 1) -> [kp, kc, n]."""
    return np.ascontiguousarray(w.T.reshape(KC, P, -1).transpose(1, 0, 2))


def _f8(a):
    return np.clip(a, -240.0, 240.0).astype(F8_NP)


def prepare_in_maps(inputs, use_fp8=True):
    x = np.asarray(inputs["x"], np.float32)
    h = np.asarray(inputs["h_prev"], np.float32)
    memory = np.asarray(inputs["memory"], np.float32)
    W_w = np.asarray(inputs["W_w"], np.float32)
    U_w = np.asarray(inputs["U_w"], np.float32)
    Q_w = np.asarray(inputs["Q_w"], np.float32)
    M_w = np.asarray(inputs["M_w"], np.float32)
    W_b = np.asarray(inputs["W_b"], np.float32)
    U_b = np.asarray(inputs["U_b"], np.float32)
    Q_b = np.asarray(inputs["Q_b"], np.float32)
    M_b = np.asarray(inputs["M_b"], np.float32)
    ln_g = np.asarray(inputs["ln_g"], np.float32)
    ln_b = np.asarray(inputs["ln_b"], np.float32)

    if use_fp8:
        Wt_h = _f8(_tile_w(W_w) * SW)
        Ut_h = _f8(_tile_w(U_w) * SW)
    else:
        Wt_h = _tile_w(W_w).astype(BF16_NP)
        Ut_h = _tile_w(U_w).astype(BF16_NP)

    # memory contraction for R = memory @ Q_w.T is over memory's axis 1
    # (HIDDEN); rows (axis 0) are the "out" dim -> same transform as W.
    if use_fp8:
        Qt_h = _f8(_tile_w(Q_w) * SW)
        mem_t = _f8(_tile_w(memory))
    else:
        Qt_h = _tile_w(Q_w).astype(BF16_NP)
        mem_t = _tile_w(memory).astype(BF16_NP)
    shared = {
        "Wt": Wt_h,
        "Ut": Ut_h,
        "Qt": Qt_h,
        "Mem": mem_t,
        "Mt": _tile_w(M_w).astype(np.float32),
        "cb": np.ascontiguousarray((W_b + U_b + Q_b)
                                   * (SW if use_fp8 else 1.0)),
        "lng": np.ascontiguousarray(ln_g),
        "lnb": np.ascontiguousarray(ln_b),
        "negmb": np.ascontiguousarray(-M_b),
        "powers": (2.0 ** np.arange(NB - 1, -1, -1)).astype(np.float32),
    }
    in_maps = []
    for i in range(NCORES):
        sl = slice(i * BC, (i + 1) * BC)
        xt = _tile_act(x[sl])
        ht = _tile_act(h[sl])
        m = dict(shared)
        m["xT"] = _f8(xt) if use_fp8 else xt.astype(BF16_NP)
        m["hT8"] = _f8(ht) if use_fp8 else ht.astype(BF16_NP)
        m["hTf"] = ht  # fp32 copy for the address logits
        in_maps.append(m)
    return in_maps


def run(inputs, trace=False, trace_cores=None):
    zero_affine = bool(
        np.all(np.asarray(inputs["ln_g"], np.float32) == 1.0)
        and np.all(np.asarray(inputs["ln_b"], np.float32) == 0.0))
    nc = _get_nc(zero_affine)
    in_maps = prepare_in_maps(inputs, use_fp8=FLAGS["use_fp8"])
    res = bass_utils.run_bass_kernel_spmd(
        nc, in_maps, core_ids=list(range(NCORES)), trace=trace,
        trace_cores=trace_cores)
    out = np.concatenate([np.asarray(r["y"], np.float32)
                          for r in res.results], axis=0)
    return out, res


def kernel(**inputs):
    out, _ = run(inputs)
    return out.astype(np.float32)


def enable_profiling():
    """Inject the missing antenv.axon_hooks shim so trace=True works, and
    neutralize the S3 artifact upload (zero-egress container)."""
    import sys
    import types
    try:
        import antenv.axon_hooks  # noqa: F401
    except ImportError:
        mod = types.ModuleType("antenv.axon_hooks")
        _hook = [None]
        mod.set_axon_ntff_profile_hook = lambda h: _hook.__setitem__(0, h)
        mod.get_axon_ntff_profile_hook = lambda: _hook[0]
        sys.modules["antenv.axon_hooks"] = mod
        from trn_agent_boot.trn_boot import _ntff_profile_via_ctypes
        mod.set_axon_ntff_profile_hook(
            _ntff_profile_via_ctypes("/opt/axon/libaxon_pjrt.so"))
    bass_utils.upload_artifacts = lambda d: "local://" + str(d)


# revision 89
# speedup vs baseline: 1.0142x; 1.0062x over previous
"""Trainium2 Bass kernel for BinaryMemoryRNN (scatter_memory).

Math (per batch row b):
    logits = h_prev @ M_w.T + M_b                 [B, 10]
    bits   = (sigmoid(logits) > 0.5) = (logits > -M_b)
    index  = sum(bits * 2^(9-i))                  [B] in [0, 1023]
    h_mem  = memory[index]
    pre    = x @ W_w.T + W_b + h_prev @ U_w.T + U_b + h_mem @ Q_w.T + Q_b
    out    = sigmoid(LayerNorm(pre) * ln_g + ln_b)

Key transforms:
  - h_mem @ Q_w.T == (memory @ Q_w.T)[index]: precompute R = memory @ Q_w.T
    + combined bias ([1024, 1024] bf16) and replace gather+matmul with a row
    gather of R feeding an add.
  - All three big GEMMs (x@W, h@U, memory@Q) run in fp8 (e4m3) with
    DoubleRow perf mode (2 fp8 weights per PE cell, K=256 per matmul,
    measured 2x over bf16). Weights are pre-scaled by SW=128 on the host
    (values ~0.02 would hit e4m3 subnormals). The scale is never undone:
    R is stored at SW-times scale, so PSUM + R-gather stays a plain
    tensor_add, and LayerNorm's scale invariance absorbs SW (only eps is
    scaled by SW^2).
  - The address logits need ~fp32 accuracy (a flipped bit selects a
    completely different memory row): computed from an fp32 copy of h
    read as float32r (fp22, 1-pass PE) in a transposed group scheme
    [10, 512] so the PE cost is moving-dominated, not LDWEIGHTS-dominated.
  - Logits/index run one tile-group ahead of the matmul pipeline; the
    output is written as bf16 and upcast on the host.

Sharding: data-parallel over batch across 8 cores (2048 rows each);
weights + memory table replicated. All operands pre-transposed/tiled on
host so the device does zero transposes:
  - activations as [kp, bt, kc, bp] tiles (K on partitions)
  - weights as [kp, kc, n] (K on partitions, contiguous rhs slices)
"""

import numpy as np
import ml_dtypes
from contextlib import ExitStack

import concourse.bass as bass
import concourse.mybir as mybir
import concourse.tile as tile
from concourse import bacc
from concourse import bass_utils

P = 128            # partitions
NCORES = 8
B = 16384          # full batch
BC = B // NCORES   # batch rows per core (2048)
BT = BC // P       # b-tiles per core (16)
KC = 8             # contraction chunks (1024 / 128)
H = 1024
NB = 10            # address bits
MEM = 1024         # memory rows
LN_EPS = 1e-5
SW = 128.0         # fp8 weight prescale

F32 = mybir.dt.float32
F32R = mybir.dt.float32r
BF16 = mybir.dt.bfloat16
F8 = mybir.dt.float8e4
I32 = mybir.dt.int32
BF16_NP = ml_dtypes.bfloat16
F8_NP = getattr(ml_dtypes, "float8_e4m3", ml_dtypes.float8_e4m3fn)

_CACHE = {}


def _bcast_ap(handle, n):
    """[n] DRAM tensor -> [P, n] AP broadcast across partitions (step 0)."""
    h = handle.ap()
    return bass.AP(tensor=h.tensor, offset=h.offset, ap=[[0, P], *list(h.ap)])


def build_nc(zero_affine=True, warmup=False, r_interleaved=False,
             use_fp8=True):
    nc = bacc.Bacc("TRN2", debug=False, enable_asserts=False,
                   num_devices=NCORES)

    act_dt = F8 if use_fp8 else BF16
    w_dt = F8 if use_fp8 else BF16
    xT = nc.dram_tensor("xT", [P, BT, KC, P], act_dt, kind="ExternalInput")
    hT8 = nc.dram_tensor("hT8", [P, BT, KC, P], act_dt, kind="ExternalInput")
    hTf = nc.dram_tensor("hTf", [P, BT, KC, P], F32R, kind="ExternalInput")
    Wt = nc.dram_tensor("Wt", [P, KC, H], w_dt, kind="ExternalInput")
    Ut = nc.dram_tensor("Ut", [P, KC, H], w_dt, kind="ExternalInput")
    rq_dt = F8 if use_fp8 else BF16
    Qt = nc.dram_tensor("Qt", [P, KC, H], rq_dt, kind="ExternalInput")
    Mem = nc.dram_tensor("Mem", [P, KC, MEM], rq_dt, kind="ExternalInput")
    Mt = nc.dram_tensor("Mt", [P, KC, NB], F32R, kind="ExternalInput")
    cb = nc.dram_tensor("cb", [H], F32, kind="ExternalInput")
    lng = nc.dram_tensor("lng", [H], F32, kind="ExternalInput")
    lnb = nc.dram_tensor("lnb", [H], F32, kind="ExternalInput")
    negmb = nc.dram_tensor("negmb", [NB], F32, kind="ExternalInput")
    powers = nc.dram_tensor("powers", [NB], F32, kind="ExternalInput")
    y = nc.dram_tensor("y", [BC, H], BF16, kind="ExternalOutput")
    R = nc.dram_tensor("Rtab", [MEM, H], BF16, kind="Internal")
    idxd = nc.dram_tensor("idxd", [BT, P], I32, kind="Internal")
    wsink_d = nc.dram_tensor("wsink", [P, 1], F32, kind="Internal")
    y_ap = y.ap()
    R_ap = R.ap()

    GROUP = 4
    DR = mybir.MatmulPerfMode.DoubleRow

    with tile.TileContext(nc) as tc, ExitStack() as ctx:
        wpool = ctx.enter_context(tc.tile_pool(name="weights", bufs=1))
        work = ctx.enter_context(tc.tile_pool(name="work", bufs=6))
        hpool = ctx.enter_context(tc.tile_pool(name="hpool", bufs=2))
        ipool = ctx.enter_context(tc.tile_pool(name="ipool", bufs=2))
        epil = ctx.enter_context(tc.tile_pool(name="epil", bufs=6))
        small = ctx.enter_context(tc.tile_pool(name="small", bufs=2 * GROUP + 2))
        psum = ctx.enter_context(tc.tile_pool(name="psum", bufs=1, space="PSUM"))

        if warmup:
            # ~5us of dummy matmuls on memset data: trips the HAM clock gate
            # to K=8/8 before real matmuls start.
            wu_l = wpool.tile([P, P], BF16)
            wu_r = wpool.tile([P, 512], BF16)
            nc.vector.memset(wu_l[:], 0)
            nc.vector.memset(wu_r[:], 0)
            ps_w = psum.tile([P, 512], F32, tag="psL", space="PSUM", bufs=2)
            for _ in range(12):
                nc.tensor.matmul(out=ps_w[:], lhsT=wu_l[:], rhs=wu_r[:],
                                 start=True, stop=True)
            wsink = wpool.tile([P, 1], F32)
            nc.vector.tensor_copy(out=wsink[:], in_=ps_w[:, 0:1])
            nc.sync.dma_start(out=wsink_d.ap()[:, :], in_=wsink[:])

        # ---- resident constants; mem/q chunked halves so R-phase matmuls
        # start early; w/u/mw on the (idle) vector queue ----
        w_sb = wpool.tile([P, KC, H], w_dt)
        u_sb = wpool.tile([P, KC, H], w_dt)
        q_sb = wpool.tile([P, KC, H], rq_dt)
        mem_sb = wpool.tile([P, KC, MEM], rq_dt)
        mw_sb = wpool.tile([P, KC, NB], F32R)
        # kc-pair quarters; mem on the scalar queue, q on sync, so both
        # streams issue in parallel and the R build starts sooner
        for kcp in range(0, KC, 2):
            nc.scalar.dma_start(out=mem_sb[:, kcp:kcp + 2],
                                in_=Mem.ap()[:, kcp:kcp + 2, :])
            nc.sync.dma_start(out=q_sb[:, kcp:kcp + 2],
                              in_=Qt.ap()[:, kcp:kcp + 2, :])
        nc.sync.dma_start(out=mw_sb[:], in_=Mt.ap()[:, :, :])
        nc.scalar.dma_start(out=w_sb[:], in_=Wt.ap()[:, :, :])
        nc.scalar.dma_start(out=u_sb[:], in_=Ut.ap()[:, :, :])

        cbb = wpool.tile([P, H], F32)
        nc.gpsimd.dma_start(out=cbb[:], in_=_bcast_ap(cb, H))
        if not zero_affine:
            gb = wpool.tile([P, H], F32)
            bb = wpool.tile([P, H], F32)
            nc.gpsimd.dma_start(out=gb[:], in_=_bcast_ap(lng, H))
            nc.gpsimd.dma_start(out=bb[:], in_=_bcast_ap(lnb, H))
        # eps scaled by SW^2: pre lives at SW-times scale in fp8 mode
        eps_val = LN_EPS * (SW * SW if use_fp8 else 1.0)
        eps = wpool.tile([P, 1], F32)
        nc.vector.memset(eps[:], eps_val)
        # transposed-group logits constants
        nmb_c = wpool.tile([NB, 1], F32)
        pw_c = wpool.tile([NB, 1], F32)
        nc.sync.dma_start(out=nmb_c[:], in_=negmb.ap()[:, None])
        nc.sync.dma_start(out=pw_c[:], in_=powers.ap()[:, None])
        ones10 = wpool.tile([NB, 1], BF16)
        nc.vector.memset(ones10[:], 1.0)

        # ---- phase 1: R = memory @ Q_w.T + combined_bias -> DRAM (bf16) ----
        # R table kept at SW-times scale (Q was pre-scaled by SW and the
        # host ships cb*SW): LayerNorm is scale-invariant, so the tile
        # epilogue never needs to unscale -- only eps is scaled by SW^2.
        for mt in range(KC):
            psA = psum.tile([P, 512], F32, tag="psA", space="PSUM", bufs=3)
            psB = psum.tile([P, 512], F32, tag="psB", space="PSUM", bufs=3)
            if use_fp8:
                for kc in range(0, KC, 2):
                    lhs = mem_sb[:, kc:kc + 2, mt * P:(mt + 1) * P]
                    nc.tensor.matmul(out=psA[:], lhsT=lhs,
                                     rhs=q_sb[:, kc:kc + 2, 0:512],
                                     start=(kc == 0), stop=(kc == KC - 2),
                                     perf_mode=DR)
                    nc.tensor.matmul(out=psB[:], lhsT=lhs,
                                     rhs=q_sb[:, kc:kc + 2, 512:1024],
                                     start=(kc == 0), stop=(kc == KC - 2),
                                     perf_mode=DR)
            else:
                for kc in range(KC):
                    lhs = mem_sb[:, kc, mt * P:(mt + 1) * P]
                    nc.tensor.matmul(out=psA[:], lhsT=lhs,
                                     rhs=q_sb[:, kc, 0:512],
                                     start=(kc == 0), stop=(kc == KC - 1))
                    nc.tensor.matmul(out=psB[:], lhsT=lhs,
                                     rhs=q_sb[:, kc, 512:1024],
                                     start=(kc == 0), stop=(kc == KC - 1))
            r_sb = work.tile([P, H], BF16, tag="rtile")
            nc.vector.tensor_add(out=r_sb[:, 0:512], in0=psA[:],
                                 in1=cbb[:, 0:512])
            nc.vector.tensor_add(out=r_sb[:, 512:1024], in0=psB[:],
                                 in1=cbb[:, 512:1024])
            nc.gpsimd.dma_start(out=R_ap[mt * P:(mt + 1) * P, :], in_=r_sb[:])

        # ---- phase 2 ----
        def logits_group(g0, gsz):
            """Transposed fp32r logits for gsz b-tiles -> per-tile idx."""
            n = gsz * P
            hfg = hpool.tile([P, gsz, KC, P], F32R, tag="hfg")
            # split along kc (the matmul iteration axis), so the first
            # psLT matmuls can start after half the transfer
            kh = KC // 2
            nc.sync.dma_start(out=hfg[:, :, 0:kh],
                              in_=hTf.ap()[:, g0:g0 + gsz, 0:kh, :])
            nc.sync.dma_start(out=hfg[:, :, kh:KC],
                              in_=hTf.ap()[:, g0:g0 + gsz, kh:KC, :])
            psLT = psum.tile([NB, 512], F32, tag="psL", space="PSUM",
                             bufs=2)
            for kc in range(KC):
                nc.tensor.matmul(out=psLT[:, 0:n],
                                 lhsT=mw_sb[:, kc, :],
                                 rhs=hfg[:, :, kc, :],
                                 start=(kc == 0), stop=(kc == KC - 1))
            bitsT = ipool.tile([NB, 512], BF16, tag="bitsT")
            nc.vector.tensor_scalar(out=bitsT[:, 0:n], in0=psLT[:, 0:n],
                                    scalar1=nmb_c[:], scalar2=pw_c[:],
                                    op0=mybir.AluOpType.is_gt,
                                    op1=mybir.AluOpType.mult)
            psI = psum.tile([1, 512], F32, tag="psL", space="PSUM", bufs=2)
            nc.tensor.matmul(out=psI[0:1, 0:n], lhsT=ones10[:],
                             rhs=bitsT[:, 0:n],
                             start=True, stop=True)
            # transpose [1, n] -> [P, gsz] via a DRAM round trip with a
            # rearranged read AP (replaces gsz PE transposes + copies)
            idxT = ipool.tile([1, 512], I32, tag="idxT")
            nc.vector.tensor_copy(out=idxT[0:1, 0:n], in_=psI[0:1, 0:n])
            nc.sync.dma_start(out=idxd.ap()[g0:g0 + gsz, :],
                              in_=idxT[0:1, 0:n])
            idxP = small.tile([P, gsz], I32, tag="idxP")
            h_ap = idxd.ap()
            nc.sync.dma_start(
                out=idxP[:],
                in_=bass.AP(tensor=h_ap.tensor, offset=g0 * P,
                            ap=[[1, P], [P, gsz]]))
            return [idxP[:, tb:tb + 1] for tb in range(gsz)]

        def stage_a(bt, idx_ap):
            xb = work.tile([P, KC, P], act_dt, tag="xb")
            hb = work.tile([P, KC, P], act_dt, tag="hb")
            nc.sync.dma_start(out=xb[:], in_=xT.ap()[:, bt, :, :])
            nc.sync.dma_start(out=hb[:], in_=hT8.ap()[:, bt, :, :])

            ps0 = psum.tile([P, 512], F32, tag="psA", space="PSUM", bufs=3)
            ps1 = psum.tile([P, 512], F32, tag="psB", space="PSUM", bufs=3)

            rg = work.tile([P, H], BF16, tag="rg")
            nc.gpsimd.indirect_dma_start(
                out=rg[:], out_offset=None, in_=R_ap[:, :],
                in_offset=bass.IndirectOffsetOnAxis(ap=idx_ap, axis=0))

            if use_fp8:
                for kc in range(0, KC, 2):
                    nc.tensor.matmul(out=ps0[:], lhsT=xb[:, kc:kc + 2, :],
                                     rhs=w_sb[:, kc:kc + 2, 0:512],
                                     start=(kc == 0), stop=False, perf_mode=DR)
                    nc.tensor.matmul(out=ps1[:], lhsT=xb[:, kc:kc + 2, :],
                                     rhs=w_sb[:, kc:kc + 2, 512:1024],
                                     start=(kc == 0), stop=False, perf_mode=DR)
                for kc in range(0, KC, 2):
                    nc.tensor.matmul(out=ps0[:], lhsT=hb[:, kc:kc + 2, :],
                                     rhs=u_sb[:, kc:kc + 2, 0:512],
                                     start=False, stop=(kc == KC - 2),
                                     perf_mode=DR)
                    nc.tensor.matmul(out=ps1[:], lhsT=hb[:, kc:kc + 2, :],
                                     rhs=u_sb[:, kc:kc + 2, 512:1024],
                                     start=False, stop=(kc == KC - 2),
                                     perf_mode=DR)
            else:
                for kc in range(KC):
                    nc.tensor.matmul(out=ps0[:], lhsT=xb[:, kc, :],
                                     rhs=w_sb[:, kc, 0:512],
                                     start=(kc == 0), stop=False)
                    nc.tensor.matmul(out=ps1[:], lhsT=xb[:, kc, :],
                                     rhs=w_sb[:, kc, 512:1024],
                                     start=(kc == 0), stop=False)
                for kc in range(KC):
                    nc.tensor.matmul(out=ps0[:], lhsT=hb[:, kc, :],
                                     rhs=u_sb[:, kc, 0:512],
                                     start=False, stop=(kc == KC - 1))
                    nc.tensor.matmul(out=ps1[:], lhsT=hb[:, kc, :],
                                     rhs=u_sb[:, kc, 512:1024],
                                     start=False, stop=(kc == KC - 1))

            pre = epil.tile([P, 2, 512], F32, tag="pre")
            nc.vector.tensor_add(out=pre[:, 0], in0=ps0[:],
                                 in1=rg[:, 0:512])
            nc.vector.tensor_add(out=pre[:, 1], in0=ps1[:],
                                 in1=rg[:, 512:1024])

            stats = small.tile([P, 2, 6], F32, tag="stats")
            mv = small.tile([P, 2], F32, tag="mv")
            nc.vector.bn_stats(out=stats[:, 0, :], in_=pre[:, 0])
            nc.vector.bn_stats(out=stats[:, 1, :], in_=pre[:, 1])
            nc.vector.bn_aggr(out=mv[:], in_=stats[:])

            if zero_affine:
                v = small.tile([P, 1], F32, tag="v")
                ri = small.tile([P, 1], I32, tag="ri")
                t = small.tile([P, 1], F32, tag="t")
                nmr = small.tile([P, 1], F32, tag="nmr")
                ry = ri[:].bitcast(F32)
                nc.vector.tensor_scalar_add(out=v[:], in0=mv[:, 1:2],
                                            scalar1=eps_val)
                nc.vector.tensor_scalar(out=ri[:], in0=v[:].bitcast(I32),
                                        scalar1=1, scalar2=None,
                                        op0=mybir.AluOpType.arith_shift_right)
                nc.vector.tensor_scalar(out=ri[:], in0=ri[:], scalar1=0,
                                        scalar2=None,
                                        op0=mybir.AluOpType.bitwise_not)
                nc.vector.tensor_scalar(out=ri[:], in0=ri[:],
                                        scalar1=0x5F3759E0, scalar2=None,
                                        op0=mybir.AluOpType.add)
                # one Newton step on the magic-constant estimate gives rstd
                # to ~0.2% -- far below the fp8 matmul noise
                for _ in range(1):
                    nc.vector.tensor_tensor(out=t[:], in0=ry, in1=ry,
                                            op=mybir.AluOpType.mult)
                    nc.vector.tensor_tensor(out=t[:], in0=t[:], in1=v[:],
                                            op=mybir.AluOpType.mult)
                    nc.vector.tensor_scalar(out=t[:], in0=t[:], scalar1=-0.5,
                                            scalar2=1.5,
                                            op0=mybir.AluOpType.mult,
                                            op1=mybir.AluOpType.add)
                    nc.vector.tensor_tensor(out=ry, in0=ry, in1=t[:],
                                            op=mybir.AluOpType.mult)
                nc.vector.scalar_tensor_tensor(out=nmr[:], in0=mv[:, 0:1],
                                               scalar=-1.0, in1=ry,
                                               op0=mybir.AluOpType.mult,
                                               op1=mybir.AluOpType.mult)
                ob = work.tile([P, H], BF16, tag="ob")
                nc.scalar.activation(out=ob[:], in_=pre[:],
                                     func=mybir.ActivationFunctionType.Sigmoid,
                                     bias=nmr[:], scale=ri[:].bitcast(F32))
                nc.scalar.dma_start(out=y_ap[bt * P:(bt + 1) * P, :], in_=ob[:])
                return None

            sd = small.tile([P, 1], F32, tag="sd")
            rstd = small.tile([P, 1], F32, tag="rstd")
            nc.scalar.activation(out=sd[:], in_=mv[:, 1:2],
                                 func=mybir.ActivationFunctionType.Sqrt,
                                 bias=eps[:], scale=1.0)
            nc.vector.reciprocal(out=rstd[:], in_=sd[:])
            return pre, mv, rstd

        def stage_b(bt, pre, mv, rstd):
            nc.vector.scalar_tensor_tensor(out=pre[:], in0=pre[:],
                                           scalar=mv[:, 0:1], in1=gb[:],
                                           op0=mybir.AluOpType.subtract,
                                           op1=mybir.AluOpType.mult)
            nc.vector.scalar_tensor_tensor(out=pre[:], in0=pre[:],
                                           scalar=rstd[:], in1=bb[:],
                                           op0=mybir.AluOpType.mult,
                                           op1=mybir.AluOpType.add)
            ob = work.tile([P, H], BF16, tag="ob")
            nc.scalar.activation(out=ob[:], in_=pre[:],
                                 func=mybir.ActivationFunctionType.Sigmoid)
            nc.scalar.dma_start(out=y_ap[bt * P:(bt + 1) * P, :], in_=ob[:])

        # logits run one group ahead of the matmul tiles so the final
        # group's drain is just tile epilogues, not the full index chain.
        # the last two groups are small so the pipeline drain is short.
        groups = [(0, 4), (4, 4), (8, 4), (12, 2), (14, 2)]
        pend = logits_group(*groups[0])
        for gi, (g0, gsz) in enumerate(groups):
            idxs = pend
            if gi + 1 < len(groups):
                pend = logits_group(*groups[gi + 1])
            if zero_affine:
                for tb in range(gsz):
                    stage_a(g0 + tb, idxs[tb])
            else:
                staged = [(g0 + tb, *stage_a(g0 + tb, idxs[tb]))
                          for tb in range(gsz)]
                for bt, pre, mv, rstd in staged:
                    stage_b(bt, pre, mv, rstd)

    nc.compile()
    return nc


import os as _os

FLAGS = {
    "warmup": bool(int(_os.environ.get("K_WARMUP", "1"))),
    "r_interleaved": bool(int(_os.environ.get("K_RINT", "0"))),
    "use_fp8": bool(int(_os.environ.get("K_FP8", "1"))),
}


def _get_nc(zero_affine=True):
    key = ("nc", zero_affine, tuple(sorted(FLAGS.items())))
    if key not in _CACHE:
        _CACHE[key] = build_nc(zero_affine, **FLAGS)
    return _CACHE[key]


def _tile_act(a):
    """[BC, 1024] -> [kp, bt, kc, bp] = a[bt*128+bp, kc*128+kp]."""
    return np.ascontiguousarray(
        a.reshape(BT, P, KC, P).transpose(3, 0, 2, 1))


def _tile_w(w):
    """[n, 1024] (n = out dim, contraction on axis 1) -> [kp, kc, n]."""
    return np.ascontiguousarray(w.T.reshape(KC, P, -1).transpose(1, 0, 2))


def _f8(a):
    return np.clip(a, -240.0, 240.0).astype(F8_NP)


def prepare_in_maps(inputs, use_fp8=True):
    x = np.asarray(inputs["x"], np.float32)
    h = np.asarray(inputs["h_prev"], np.float32)
    memory = np.asarray(inputs["memory"], np.float32)
    W_w = np.asarray(inputs["W_w"], np.float32)
    U_w = np.asarray(inputs["U_w"], np.float32)
    Q_w = np.asarray(inputs["Q_w"], np.float32)
    M_w = np.asarray(inputs["M_w"], np.float32)
    W_b = np.asarray(inputs["W_b"], np.float32)
    U_b = np.asarray(inputs["U_b"], np.float32)
    Q_b = np.asarray(inputs["Q_b"], np.float32)
    M_b = np.asarray(inputs["M_b"], np.float32)
    ln_g = np.asarray(inputs["ln_g"], np.float32)
    ln_b = np.asarray(inputs["ln_b"], np.float32)

    if use_fp8:
        Wt_h = _f8(_tile_w(W_w) * SW)
        Ut_h = _f8(_tile_w(U_w) * SW)
    else:
        Wt_h = _tile_w(W_w).astype(BF16_NP)
        Ut_h = _tile_w(U_w).astype(BF16_NP)

    # memory contraction for R = memory @ Q_w.T is over memory's axis 1
    # (HIDDEN); rows (axis 0) are the "out" dim -> same transform as W.
    if use_fp8:
        Qt_h = _f8(_tile_w(Q_w) * SW)
        mem_t = _f8(_tile_w(memory))
    else:
        Qt_h = _tile_w(Q_w).astype(BF16_NP)
        mem_t = _tile_w(memory).astype(BF16_NP)
    shared = {
        "Wt": Wt_h,
        "Ut": Ut_h,
        "Qt": Qt_h,
        "Mem": mem_t,
        "Mt": _tile_w(M_w).astype(np.float32),
        "cb": np.ascontiguousarray((W_b + U_b + Q_b)
                                   * (SW if use_fp8 else 1.0)),
        "lng": np.ascontiguousarray(ln_g),
        "lnb": np.ascontiguousarray(ln_b),
        "negmb": np.ascontiguousarray(-M_b),
        "powers": (2.0 ** np.arange(NB - 1, -1, -1)).astype(np.float32),
    }
    in_maps = []
    for i in range(NCORES):
        sl = slice(i * BC, (i + 1) * BC)
        xt = _tile_act(x[sl])
        ht = _tile_act(h[sl])
        m = dict(shared)
        m["xT"] = _f8(xt) if use_fp8 else xt.astype(BF16_NP)
        m["hT8"] = _f8(ht) if use_fp8 else ht.astype(BF16_NP)
        m["hTf"] = ht  # fp32 copy for the address logits
        in_maps.append(m)
    return in_maps


def run(inputs, trace=False, trace_cores=None):
    zero_affine = bool(
        np.all(np.asarray(inputs["ln_g"], np.float32) == 1.0)
        and np.all(np.asarray(inputs["ln_b"], np.float32) == 0.0))
    nc = _get_nc(zero_affine)
    in_maps = prepare_in_maps(inputs, use_fp8=FLAGS["use_fp8"])
    res = bass_utils.run_bass_kernel_spmd(
        nc, in_maps, core_ids=list(range(NCORES)), trace=trace,
        trace_cores=trace_cores)
    out = np.concatenate([np.asarray(r["y"], np.float32)
                          for r in res.results], axis=0)
    return out, res


def kernel(**inputs):
    out, _ = run(inputs)
    return out.astype(np.float32)


def enable_profiling():
    """Inject the missing antenv.axon_hooks shim so trace=True works, and
    neutralize the S3 artifact upload (zero-egress container)."""
    import sys
    import types
    try:
        import antenv.axon_hooks  # noqa: F401
    except ImportError:
        mod = types.ModuleType("antenv.axon_hooks")
        _hook = [None]
        mod.set_axon_ntff_profile_hook = lambda h: _hook.__setitem__(0, h)
        mod.get_axon_ntff_profile_hook = lambda: _hook[0]
        sys.modules["antenv.axon_hooks"] = mod
        from trn_agent_boot.trn_boot import _ntff_profile_via_ctypes
        mod.set_axon_ntff_profile_hook(
            _ntff_profile_via_ctypes("/opt/axon/libaxon_pjrt.so"))
    bass_utils.upload_artifacts = lambda d: "local://" + str(d)
